# revision 2
# baseline (speedup 1.0000x reference)
"""Trainium2 Bass kernel: HLIF spiking layer forward (LIF with soft reset).

Reference semantics per neuron, scanned over T:
    v' = v * decay + x_t ;  s = (v' - vth > 0) ;  v'' = v' - s * vth

Device formulation (z-space: z = v/vth - 1, so the spike test is z > 0):

    z_t = p_{t-1} + xs_t                  TT add                (DVE)
    s_t = Sign(z_t)  in {-1,0,1}, bf16    spike                 (ScalarE ACT)
    p_t = ((z_t - [z_t>0]) + 1)*dec - 1   custom 5-stage op     (DVE)

with p_init = -1 (a single memset) and xs = x / vth prescaled on host.
The (dec - 1) shift of the state absorbs the threshold subtraction, so
each timestep costs exactly two DVE sweeps; the spike compare runs on the
otherwise-idle scalar engine.

Spikes are bit-packed on device: for each group of 8 timesteps the sign
tiles accumulate into PSUM through identity matmuls with weights
2^(k-1)*I (k = t%8) on the otherwise-idle PE; since
sum_k 2^(k-1)*(sign_k + 1) = packed_byte, the constant offset 127.5 is
added at the ACT PSUM->uint8 copy (bias).  One 128 KiB u8 DMA out per
group -> output per core [T/8, 128, 1024] u8, unpacked to bits on host.
This cuts spike-store DMA ~16x vs bf16 spikes.

Sharding: data-parallel over batch B=16 across 8 cores; the 2 local
batch items are fused along tile columns ([128, 1024] tiles; cols
[0:512] = b0, [512:1024] = b1).  vth/decay are replicated.

Measured (honest tc.For_i repeat timing, per full scan per core):
~96-106 us vs ~148 us for the previous 3-DVE-sweep kernel.
"""

import numpy as np

B, T, C, H, W = 16, 32, 64, 32, 32
VTH_M, VTH_S, DECAY_M, DECAY_S = 0.5, 0.1, 2.0, 0.1
N_CORES = 8
B_LOC = B // N_CORES          # 2 batch items per core
P = 128                       # SBUF partitions
CHW = C * H * W               # 65536 neurons
FD = CHW // P                 # 512
FDB = B_LOC * FD              # 1024 (both batch items along columns)
NG = T // 8                   # 4 bit-pack groups

_STATE: dict = {}


# --------------------------------------------------------------------------
# Custom DVE op (registered once per process)
# --------------------------------------------------------------------------

def _get_ops():
    if "ops" in _STATE:
        return _STATE["ops"]
    from concourse import dve_ops
    from concourse.dve_spec import Spec, Src0, Src1, Zero, One, lower, _has_src1
    from concourse.dve_uop import DveOpSpec

    def register(name, spec):
        for op in dve_ops.OPS:
            if op.name == name:
                return op
        row = dve_ops._CUSTOM_DVE_ROW_BASE + len(dve_ops.OPS)
        shas = {}
        for ver in ("v3", "v4"):
            s = DveOpSpec(
                name=name, opcode=row, uops=lower(spec, ver=ver),
                rd1_en=_has_src1(spec),
            )
            shas[ver] = s.sha(ver)
        op = dve_ops.DveOp(name, spec, subdim=False, uops_sha=shas)
        dve_ops.OPS.append(op)
        dve_ops._SUB_OPCODE_FOR_NAME[name] = row
        dve_ops.CUSTOM_DVE_SPECS[name] = spec
        return op

    # p' = ((z - (z>0)) + 1) * d - 1   (soft reset + leak + state shift)
    lif_pd = register(
        "LIF_PD",
        Spec(
            body=((Src0 - (Src0 > Zero)) + One) * Src1 - One,
            reference=lambda in0, in1, s0, s1, imm2: (
                ((in0.astype(np.float32) - (in0 > 0.0)) + 1.0) * in1 - 1.0
            ).astype(np.float32),
        ),
    )
    _STATE["ops"] = (lif_pd,)
    return _STATE["ops"]


# --------------------------------------------------------------------------
# Device kernel build
# --------------------------------------------------------------------------

def _emit_scan(nc, tc, pools, tiles, mybir):
    """One full T-step scan (the per-iteration body)."""
    import concourse.bass  # noqa: F401
    (lif_pd,) = _get_ops()
    f32 = mybir.dt.float32
    bf16 = mybir.dt.bfloat16
    u8 = mybir.dt.uint8
    Act = mybir.ActivationFunctionType
    xp, wp, zp, sp, up, ps = pools
    xs_d, s_d, dec, eyes, out_slot = tiles

    p = wp.tile([P, FDB], f32, name="p", tag="p")
    nc.vector.memset(p, -1.0)
    psums = [None, None]
    ut = None
    for t in range(T):
        k = t % 8
        xt = xp.tile([P, FDB], f32, name=f"x{t}", tag="x")
        nc.sync.dma_start(xt, xs_d[t])
        if k == 0:
            ut = up.tile([P, FDB], u8, name=f"u{t}", tag="u")

        z = zp.tile([P, FDB], f32, name=f"z{t}", tag="z")
        nc.vector.tensor_tensor(z, p, xt, mybir.AluOpType.add)

        st = sp.tile([P, FDB], bf16, name=f"s{t}", tag="s")
        nc.scalar.activation(st, z, Act.Sign)

        if t < T - 1:
            pn = wp.tile([P, FDB], f32, name=f"pn{t}", tag="p")
            nc.vector._custom_dve(lif_pd, out=pn, in0=z, in1=dec)
            p = pn

        for h in range(B_LOC):
            if k == 0:
                psums[h] = ps.tile([P, FD], f32, name=f"ps{t}_{h}", tag=f"ps{h}")
            nc.tensor.matmul(
                psums[h], eyes[k], st[:, h * FD:(h + 1) * FD],
                start=(k == 0), stop=(k == 7))

        if k == 7:
            for h in range(B_LOC):
                nc.scalar.activation(
                    ut[:, h * FD:(h + 1) * FD], psums[h],
                    Act.Copy, bias=127.5)
            nc.sync.dma_start(s_d[out_slot, t // 8], ut)


def _build_nc(loop_reps=None):
    """loop_reps=None: single-shot kernel.  loop_reps=R: the scan body is
    wrapped in a tc.For_i hardware loop (used only for timing)."""
    import concourse.bacc as bacc
    import concourse.mybir as mybir
    import concourse.bass as bass
    from concourse.tile import TileContext

    f32 = mybir.dt.float32
    bf16 = mybir.dt.bfloat16
    u8 = mybir.dt.uint8

    nc = bacc.Bacc(trn_type="TRN2")
    xs_d = nc.dram_tensor("xs", [T, P, FDB], f32, kind="ExternalInput")
    dec_d = nc.dram_tensor("decay2", [P, FDB], f32, kind="ExternalInput")
    eye_d = nc.dram_tensor("eye8", [8, P, P], bf16, kind="ExternalInput")
    s_d = nc.dram_tensor("spk", [1, NG, P, FDB], u8, kind="ExternalOutput")

    with TileContext(nc) as tc:
        with tc.tile_pool(name="pp", bufs=1) as pp, \
             tc.tile_pool(name="xp", bufs=6) as xp, \
             tc.tile_pool(name="wp", bufs=3) as wp, \
             tc.tile_pool(name="zp", bufs=3) as zp, \
             tc.tile_pool(name="sp", bufs=4) as sp, \
             tc.tile_pool(name="up", bufs=2) as up, \
             tc.tile_pool(name="ps", bufs=3, space=bass.MemorySpace.PSUM) as ps:

            dec = pp.tile([P, FDB], f32, name="dec", tag="dec")
            nc.sync.dma_start(dec, dec_d[:, :])
            eyes = []
            for k in range(8):
                et = pp.tile([P, P], bf16, name=f"eye{k}", tag=f"eye{k}")
                nc.sync.dma_start(et, eye_d[k])
                eyes.append(et)

            pools = (xp, wp, zp, sp, up, ps)
            tiles = (xs_d, s_d, dec, eyes, 0)
            if loop_reps is None:
                _emit_scan(nc, tc, pools, tiles, mybir)
            else:
                with tc.For_i(0, loop_reps) as _i:
                    _emit_scan(nc, tc, pools, tiles, mybir)
    nc.finalize()
    return nc


def _get_nc():
    nc = _STATE.get("nc")
    if nc is None:
        nc = _build_nc()
        _STATE["nc"] = nc
    return nc


# --------------------------------------------------------------------------
# Runner (cached jit; NEFF runs via PJRT, same path as run_bass_kernel_spmd
# under axon, but keeps the executable + device inputs reusable)
# --------------------------------------------------------------------------

def _make_runner(nc):
    import jax
    from jax.sharding import Mesh, PartitionSpec
    from jax.experimental.shard_map import shard_map
    import concourse.mybir as mybir
    from concourse import bass2jax

    bass2jax.install_neuronx_cc_hook()

    partition_name = nc.partition_id_tensor.name if nc.partition_id_tensor else None
    in_names, out_names, out_avals, zero_outs = [], [], [], []
    for alloc in nc.m.functions[0].allocations:
        if not isinstance(alloc, mybir.MemoryLocationSet):
            continue
        name = alloc.memorylocations[0].name
        if alloc.kind == "ExternalInput":
            if name != partition_name:
                in_names.append(name)
        elif alloc.kind == "ExternalOutput":
            shape = tuple(alloc.tensor_shape)
            dtype = mybir.dt.np(alloc.dtype)
            out_names.append(name)
            out_avals.append(jax.core.ShapedArray(shape, dtype))
            zero_outs.append(np.zeros(shape, dtype))
    n_params = len(in_names)
    n_outs = len(out_avals)
    all_in_names = list(in_names) + list(out_names)
    if partition_name is not None:
        all_in_names.append(partition_name)

    def _body(*args):
        operands = list(args)
        if partition_name is not None:
            operands.append(bass2jax.partition_id_tensor())
        outs = bass2jax._bass_exec_p.bind(
            *operands,
            out_avals=tuple(out_avals),
            in_names=tuple(all_in_names),
            out_names=tuple(out_names),
            lowering_input_output_aliases=(),
            sim_require_finite=True,
            sim_require_nnan=True,
            nc=nc,
        )
        return tuple(outs)

    devices = jax.devices()[:N_CORES]
    mesh = Mesh(np.asarray(devices), ("core",))
    in_specs = (PartitionSpec("core"),) * (n_params + n_outs)
    out_specs = (PartitionSpec("core"),) * n_outs
    sharded = jax.jit(
        shard_map(_body, mesh=mesh, in_specs=in_specs, out_specs=out_specs,
                  check_rep=False),
        keep_unused=True,
    )

    from jax.sharding import NamedSharding
    zero_sharding = NamedSharding(mesh, PartitionSpec("core"))
    zero_cache = []

    def run(concat_inputs_by_name):
        if not zero_cache:
            zero_cache.extend(
                jax.device_put(
                    np.zeros((N_CORES * z.shape[0], *z.shape[1:]), z.dtype),
                    zero_sharding,
                )
                for z in zero_outs
            )
        args = [concat_inputs_by_name[n] for n in in_names]
        args += zero_cache
        outs = sharded(*args)
        return outs, out_names

    run.mesh = mesh
    run.in_names = in_names
    run.out_names = out_names
    return run


# --------------------------------------------------------------------------
# Host wrapper
# --------------------------------------------------------------------------

def _prep_inputs(x, vth_raw, decay_raw):
    import ml_dtypes
    x = np.asarray(x, dtype=np.float32)
    vth_raw = np.asarray(vth_raw, dtype=np.float32)
    decay_raw = np.asarray(decay_raw, dtype=np.float32)

    vth64 = np.logaddexp(0.0, vth_raw.astype(np.float64) * VTH_S + VTH_M) + 0.01
    dec64 = 1.0 / (1.0 + np.exp(-(decay_raw.astype(np.float64) * DECAY_S + DECAY_M)))
    dec = np.clip(dec64, 0.0, 0.99).astype(np.float32)
    ivth = (1.0 / vth64).astype(np.float32)

    xs = x * ivth[None, None]                       # (B,T,C,H,W) f32
    # per-core layout [T, P, FDB]: cols [b*FD:(b+1)*FD] = batch item b
    xs_bt = xs.reshape(N_CORES, B_LOC, T, P, FD)
    xs_rs = np.ascontiguousarray(xs_bt.transpose(0, 2, 3, 1, 4))
    xs_rs = xs_rs.reshape(N_CORES, T, P, FDB)
    dec_rs = dec.reshape(P, FD)
    dec2 = np.concatenate([dec_rs, dec_rs], axis=1)  # [P, FDB]

    eye8 = np.zeros((8, P, P), np.float32)
    for k in range(8):
        np.fill_diagonal(eye8[k], 2.0 ** (k - 1))
    eye8 = eye8.astype(ml_dtypes.bfloat16)

    return {
        "xs": xs_rs.reshape(N_CORES * T, P, FDB),
        "decay2": np.concatenate([dec2] * N_CORES, axis=0),
        "eye8": np.concatenate([eye8] * N_CORES, axis=0),
    }


def _unpack(spk_u8):
    """(N_CORES, NG, P, FDB) u8 -> (B,T,C,H,W) f32 spike tensor."""
    v = spk_u8.reshape(N_CORES, NG, P, B_LOC, FD)
    bits = np.unpackbits(v[..., None], axis=-1, bitorder="little")
    # (cores, NG, P, B_LOC, FD, 8) -> (cores, B_LOC, NG, 8, P, FD)
    bits = bits.transpose(0, 3, 1, 5, 2, 4)
    return bits.reshape(B, T, C, H, W).astype(np.float32)


def kernel(x, vth_raw, decay_raw):
    import jax
    from jax.sharding import NamedSharding, PartitionSpec

    concat = _prep_inputs(x, vth_raw, decay_raw)
    nc = _get_nc()
    run = _STATE.get("run")
    if run is None:
        run = _make_runner(nc)
        _STATE["run"] = run
    sh = NamedSharding(run.mesh, PartitionSpec("core"))
    dev_in = {n: jax.device_put(concat[n], sh) for n in run.in_names}
    outs, names = run(dev_in)
    jax.block_until_ready(outs)
    res = {n: np.asarray(o) for n, o in zip(names, outs)}
    spk = res["spk"].reshape(N_CORES, 1, NG, P, FDB)[:, 0]
    return _unpack(spk)


# --------------------------------------------------------------------------
# Honest device timing: tc.For_i hardware loop re-executes the same scan
# instructions R times (immune to dead-code elimination across repeats);
# min-wall delta between R=16 and R=1024 makes device time dominate the
# quantized transport floor of this environment.
# --------------------------------------------------------------------------

def measure_hw_ns(x, vth_raw, decay_raw, r_lo=16, r_hi=1024, n_calls=12):
    import time
    import jax
    from jax.sharding import NamedSharding, PartitionSpec

    concat = _prep_inputs(x, vth_raw, decay_raw)
    mins = {}
    for R in (r_lo, r_hi):
        nc = _build_nc(loop_reps=R)
        run = _make_runner(nc)
        sh = NamedSharding(run.mesh, PartitionSpec("core"))
        dev_in = {n: jax.device_put(concat[n], sh) for n in run.in_names}
        outs, _ = run(dev_in)
        jax.block_until_ready(outs)
        ts = []
        for _ in range(n_calls):
            t0 = time.perf_counter()
            outs, _ = run(dev_in)
            jax.block_until_ready(outs)
            ts.append(time.perf_counter() - t0)
        mins[R] = min(ts)
        print(f"  R={R}: min={min(ts)*1e3:.2f} ms  "
              f"all={[f'{t*1e3:.1f}' for t in sorted(ts)]}")
    ns = (mins[r_hi] - mins[r_lo]) / (r_hi - r_lo) * 1e9
    return ns, mins


# revision 3
# speedup vs baseline: 1.0830x; 1.0830x over previous
"""Trainium2 Bass kernel: HLIF spiking layer forward (LIF with soft reset).

Reference semantics per neuron, scanned over T:
    v' = v * decay + x_t ;  s = (v' - vth > 0) ;  v'' = v' - s * vth

Device formulation (z-space: z = v/vth - 1, so the spike test is z > 0):

    z_t = p_{t-1} + xs_t                  TT add                (DVE)
    s_t = Sign(z_t)  in {-1,0,1}, bf16    spike                 (ScalarE ACT)
    p_t = ((z_t - [z_t>0]) + 1)*dec - 1   custom 5-stage op     (DVE)

with p_init = -1 (a single memset) and xs = x / vth prescaled on host.
The (dec - 1) shift of the state absorbs the threshold subtraction, so
each timestep costs exactly two DVE sweeps; the spike compare runs on the
otherwise-idle scalar engine.

Spikes are bit-packed on device: for each group of 8 timesteps the sign
tiles accumulate into PSUM through identity matmuls with weights
2^(k-1)*I (k = t%8) on the otherwise-idle PE; since
sum_k 2^(k-1)*(sign_k + 1) = packed_byte, the constant offset 127.5 is
added at the ACT PSUM->uint8 copy (bias).  One 128 KiB u8 DMA out per
group -> output per core [T/8, 128, 1024] u8, unpacked to bits on host.
This cuts spike-store DMA ~16x vs bf16 spikes.

Sharding: data-parallel over batch B=16 across 8 cores; the 2 local
batch items are fused along tile columns ([128, 1024] tiles; cols
[0:512] = b0, [512:1024] = b1).  vth/decay are replicated.

Measured (honest tc.For_i repeat timing, per full scan per core):
~96-106 us vs ~148 us for the previous 3-DVE-sweep kernel.
"""

import numpy as np

B, T, C, H, W = 16, 32, 64, 32, 32
VTH_M, VTH_S, DECAY_M, DECAY_S = 0.5, 0.1, 2.0, 0.1
N_CORES = 8
B_LOC = B // N_CORES          # 2 batch items per core
P = 128                       # SBUF partitions
CHW = C * H * W               # 65536 neurons
FD = CHW // P                 # 512
FDB = B_LOC * FD              # 1024 (both batch items along columns)
NG = T // 8                   # 4 bit-pack groups

_STATE: dict = {}


# --------------------------------------------------------------------------
# Custom DVE op (registered once per process)
# --------------------------------------------------------------------------

def _get_ops():
    if "ops" in _STATE:
        return _STATE["ops"]
    from concourse import dve_ops
    from concourse.dve_spec import Spec, Src0, Src1, Zero, One, lower, _has_src1
    from concourse.dve_uop import DveOpSpec

    def register(name, spec):
        for op in dve_ops.OPS:
            if op.name == name:
                return op
        row = dve_ops._CUSTOM_DVE_ROW_BASE + len(dve_ops.OPS)
        shas = {}
        for ver in ("v3", "v4"):
            s = DveOpSpec(
                name=name, opcode=row, uops=lower(spec, ver=ver),
                rd1_en=_has_src1(spec),
            )
            shas[ver] = s.sha(ver)
        op = dve_ops.DveOp(name, spec, subdim=False, uops_sha=shas)
        dve_ops.OPS.append(op)
        dve_ops._SUB_OPCODE_FOR_NAME[name] = row
        dve_ops.CUSTOM_DVE_SPECS[name] = spec
        return op

    # p' = ((z - (z>0)) + 1) * d - 1   (soft reset + leak + state shift)
    lif_pd = register(
        "LIF_PD",
        Spec(
            body=((Src0 - (Src0 > Zero)) + One) * Src1 - One,
            reference=lambda in0, in1, s0, s1, imm2: (
                ((in0.astype(np.float32) - (in0 > 0.0)) + 1.0) * in1 - 1.0
            ).astype(np.float32),
        ),
    )
    _STATE["ops"] = (lif_pd,)
    return _STATE["ops"]


# --------------------------------------------------------------------------
# Device kernel build
# --------------------------------------------------------------------------

def _emit_scan(nc, tc, pools, tiles, mybir):
    """One full T-step scan (the per-iteration body)."""
    import concourse.bass  # noqa: F401
    (lif_pd,) = _get_ops()
    f32 = mybir.dt.float32
    bf16 = mybir.dt.bfloat16
    u8 = mybir.dt.uint8
    Act = mybir.ActivationFunctionType
    xp, wp, zp, sp, up, ps = pools
    xs_d, s_d, dec, eyes, out_slot = tiles

    p = wp.tile([P, FDB], f32, name="p", tag="p")
    nc.vector.memset(p, -1.0)
    psums = [None, None]
    ut = None
    for t in range(T):
        k = t % 8
        xt = xp.tile([P, FDB], f32, name=f"x{t}", tag="x")
        nc.sync.dma_start(xt, xs_d[t])
        if k == 0:
            ut = up.tile([P, FDB], u8, name=f"u{t}", tag="u")

        # DVE ops are issued in column halves: dependent ops end up two
        # issue slots apart, hiding the dependent-issue stall (~10 us/scan
        # measured vs full-width ops).
        z = zp.tile([P, FDB], f32, name=f"z{t}", tag="z")
        for g in range(2):
            cg = slice(g * FD, (g + 1) * FD)
            nc.vector.tensor_tensor(
                z[:, cg], p[:, cg], xt[:, cg], mybir.AluOpType.add)

        st = sp.tile([P, FDB], bf16, name=f"s{t}", tag="s")
        nc.scalar.activation(st, z, Act.Sign)

        if t < T - 1:
            pn = wp.tile([P, FDB], f32, name=f"pn{t}", tag="p")
            for g in range(2):
                cg = slice(g * FD, (g + 1) * FD)
                nc.vector._custom_dve(
                    lif_pd, out=pn[:, cg], in0=z[:, cg], in1=dec[:, cg])
            p = pn

        for h in range(B_LOC):
            if k == 0:
                psums[h] = ps.tile([P, FD], f32, name=f"ps{t}_{h}", tag=f"ps{h}")
            nc.tensor.matmul(
                psums[h], eyes[k], st[:, h * FD:(h + 1) * FD],
                start=(k == 0), stop=(k == 7))

        if k == 7:
            for h in range(B_LOC):
                nc.scalar.activation(
                    ut[:, h * FD:(h + 1) * FD], psums[h],
                    Act.Copy, bias=127.5)
            nc.sync.dma_start(s_d[out_slot, t // 8], ut)


def _build_nc(loop_reps=None):
    """loop_reps=None: single-shot kernel.  loop_reps=R: the scan body is
    wrapped in a tc.For_i hardware loop (used only for timing)."""
    import concourse.bacc as bacc
    import concourse.mybir as mybir
    import concourse.bass as bass
    from concourse.tile import TileContext

    f32 = mybir.dt.float32
    bf16 = mybir.dt.bfloat16
    u8 = mybir.dt.uint8

    nc = bacc.Bacc(trn_type="TRN2")
    xs_d = nc.dram_tensor("xs", [T, P, FDB], f32, kind="ExternalInput")
    dec_d = nc.dram_tensor("decay2", [P, FDB], f32, kind="ExternalInput")
    eye_d = nc.dram_tensor("eye8", [8, P, P], bf16, kind="ExternalInput")
    s_d = nc.dram_tensor("spk", [1, NG, P, FDB], u8, kind="ExternalOutput")

    with TileContext(nc) as tc:
        with tc.tile_pool(name="pp", bufs=1) as pp, \
             tc.tile_pool(name="xp", bufs=6) as xp, \
             tc.tile_pool(name="wp", bufs=3) as wp, \
             tc.tile_pool(name="zp", bufs=3) as zp, \
             tc.tile_pool(name="sp", bufs=4) as sp, \
             tc.tile_pool(name="up", bufs=2) as up, \
             tc.tile_pool(name="ps", bufs=3, space=bass.MemorySpace.PSUM) as ps:

            dec = pp.tile([P, FDB], f32, name="dec", tag="dec")
            nc.sync.dma_start(dec, dec_d[:, :])
            eyes = []
            for k in range(8):
                et = pp.tile([P, P], bf16, name=f"eye{k}", tag=f"eye{k}")
                nc.sync.dma_start(et, eye_d[k])
                eyes.append(et)

            pools = (xp, wp, zp, sp, up, ps)
            tiles = (xs_d, s_d, dec, eyes, 0)
            if loop_reps is None:
                _emit_scan(nc, tc, pools, tiles, mybir)
            else:
                with tc.For_i(0, loop_reps) as _i:
                    _emit_scan(nc, tc, pools, tiles, mybir)
    nc.finalize()
    return nc


def _get_nc():
    nc = _STATE.get("nc")
    if nc is None:
        nc = _build_nc()
        _STATE["nc"] = nc
    return nc


# --------------------------------------------------------------------------
# Runner (cached jit; NEFF runs via PJRT, same path as run_bass_kernel_spmd
# under axon, but keeps the executable + device inputs reusable)
# --------------------------------------------------------------------------

def _make_runner(nc):
    import jax
    from jax.sharding import Mesh, PartitionSpec
    from jax.experimental.shard_map import shard_map
    import concourse.mybir as mybir
    from concourse import bass2jax

    bass2jax.install_neuronx_cc_hook()

    partition_name = nc.partition_id_tensor.name if nc.partition_id_tensor else None
    in_names, out_names, out_avals, zero_outs = [], [], [], []
    for alloc in nc.m.functions[0].allocations:
        if not isinstance(alloc, mybir.MemoryLocationSet):
            continue
        name = alloc.memorylocations[0].name
        if alloc.kind == "ExternalInput":
            if name != partition_name:
                in_names.append(name)
        elif alloc.kind == "ExternalOutput":
            shape = tuple(alloc.tensor_shape)
            dtype = mybir.dt.np(alloc.dtype)
            out_names.append(name)
            out_avals.append(jax.core.ShapedArray(shape, dtype))
            zero_outs.append(np.zeros(shape, dtype))
    n_params = len(in_names)
    n_outs = len(out_avals)
    all_in_names = list(in_names) + list(out_names)
    if partition_name is not None:
        all_in_names.append(partition_name)

    def _body(*args):
        operands = list(args)
        if partition_name is not None:
            operands.append(bass2jax.partition_id_tensor())
        outs = bass2jax._bass_exec_p.bind(
            *operands,
            out_avals=tuple(out_avals),
            in_names=tuple(all_in_names),
            out_names=tuple(out_names),
            lowering_input_output_aliases=(),
            sim_require_finite=True,
            sim_require_nnan=True,
            nc=nc,
        )
        return tuple(outs)

    devices = jax.devices()[:N_CORES]
    mesh = Mesh(np.asarray(devices), ("core",))
    in_specs = (PartitionSpec("core"),) * (n_params + n_outs)
    out_specs = (PartitionSpec("core"),) * n_outs
    sharded = jax.jit(
        shard_map(_body, mesh=mesh, in_specs=in_specs, out_specs=out_specs,
                  check_rep=False),
        keep_unused=True,
    )

    from jax.sharding import NamedSharding
    zero_sharding = NamedSharding(mesh, PartitionSpec("core"))
    zero_cache = []

    def run(concat_inputs_by_name):
        if not zero_cache:
            zero_cache.extend(
                jax.device_put(
                    np.zeros((N_CORES * z.shape[0], *z.shape[1:]), z.dtype),
                    zero_sharding,
                )
                for z in zero_outs
            )
        args = [concat_inputs_by_name[n] for n in in_names]
        args += zero_cache
        outs = sharded(*args)
        return outs, out_names

    run.mesh = mesh
    run.in_names = in_names
    run.out_names = out_names
    return run


# --------------------------------------------------------------------------
# Host wrapper
# --------------------------------------------------------------------------

def _prep_inputs(x, vth_raw, decay_raw):
    import ml_dtypes
    x = np.asarray(x, dtype=np.float32)
    vth_raw = np.asarray(vth_raw, dtype=np.float32)
    decay_raw = np.asarray(decay_raw, dtype=np.float32)

    vth64 = np.logaddexp(0.0, vth_raw.astype(np.float64) * VTH_S + VTH_M) + 0.01
    dec64 = 1.0 / (1.0 + np.exp(-(decay_raw.astype(np.float64) * DECAY_S + DECAY_M)))
    dec = np.clip(dec64, 0.0, 0.99).astype(np.float32)
    ivth = (1.0 / vth64).astype(np.float32)

    xs = x * ivth[None, None]                       # (B,T,C,H,W) f32
    # per-core layout [T, P, FDB]: cols [b*FD:(b+1)*FD] = batch item b
    xs_bt = xs.reshape(N_CORES, B_LOC, T, P, FD)
    xs_rs = np.ascontiguousarray(xs_bt.transpose(0, 2, 3, 1, 4))
    xs_rs = xs_rs.reshape(N_CORES, T, P, FDB)
    dec_rs = dec.reshape(P, FD)
    dec2 = np.concatenate([dec_rs, dec_rs], axis=1)  # [P, FDB]

    eye8 = np.zeros((8, P, P), np.float32)
    for k in range(8):
        np.fill_diagonal(eye8[k], 2.0 ** (k - 1))
    eye8 = eye8.astype(ml_dtypes.bfloat16)

    return {
        "xs": xs_rs.reshape(N_CORES * T, P, FDB),
        "decay2": np.concatenate([dec2] * N_CORES, axis=0),
        "eye8": np.concatenate([eye8] * N_CORES, axis=0),
    }


def _unpack(spk_u8):
    """(N_CORES, NG, P, FDB) u8 -> (B,T,C,H,W) f32 spike tensor."""
    v = spk_u8.reshape(N_CORES, NG, P, B_LOC, FD)
    bits = np.unpackbits(v[..., None], axis=-1, bitorder="little")
    # (cores, NG, P, B_LOC, FD, 8) -> (cores, B_LOC, NG, 8, P, FD)
    bits = bits.transpose(0, 3, 1, 5, 2, 4)
    return bits.reshape(B, T, C, H, W).astype(np.float32)


def kernel(x, vth_raw, decay_raw):
    import jax
    from jax.sharding import NamedSharding, PartitionSpec

    concat = _prep_inputs(x, vth_raw, decay_raw)
    nc = _get_nc()
    run = _STATE.get("run")
    if run is None:
        run = _make_runner(nc)
        _STATE["run"] = run
    sh = NamedSharding(run.mesh, PartitionSpec("core"))
    dev_in = {n: jax.device_put(concat[n], sh) for n in run.in_names}
    outs, names = run(dev_in)
    jax.block_until_ready(outs)
    res = {n: np.asarray(o) for n, o in zip(names, outs)}
    spk = res["spk"].reshape(N_CORES, 1, NG, P, FDB)[:, 0]
    return _unpack(spk)


# --------------------------------------------------------------------------
# Honest device timing: tc.For_i hardware loop re-executes the same scan
# instructions R times (immune to dead-code elimination across repeats);
# min-wall delta between R=16 and R=1024 makes device time dominate the
# quantized transport floor of this environment.
# --------------------------------------------------------------------------

def measure_hw_ns(x, vth_raw, decay_raw, r_lo=16, r_hi=1024, n_calls=12):
    import time
    import jax
    from jax.sharding import NamedSharding, PartitionSpec

    concat = _prep_inputs(x, vth_raw, decay_raw)
    mins = {}
    for R in (r_lo, r_hi):
        nc = _build_nc(loop_reps=R)
        run = _make_runner(nc)
        sh = NamedSharding(run.mesh, PartitionSpec("core"))
        dev_in = {n: jax.device_put(concat[n], sh) for n in run.in_names}
        outs, _ = run(dev_in)
        jax.block_until_ready(outs)
        ts = []
        for _ in range(n_calls):
            t0 = time.perf_counter()
            outs, _ = run(dev_in)
            jax.block_until_ready(outs)
            ts.append(time.perf_counter() - t0)
        mins[R] = min(ts)
        print(f"  R={R}: min={min(ts)*1e3:.2f} ms  "
              f"all={[f'{t*1e3:.1f}' for t in sorted(ts)]}")
    ns = (mins[r_hi] - mins[r_lo]) / (r_hi - r_lo) * 1e9
    return ns, mins


# revision 5
# speedup vs baseline: 1.1520x; 1.0638x over previous
"""Trainium2 Bass kernel: HLIF spiking layer forward (LIF with soft reset).

Reference semantics per neuron, scanned over T:
    v' = v * decay + x_t ;  s = (v' - vth > 0) ;  v'' = v' - s * vth

Device formulation (z-space: z = v/vth - 1, so the spike test is z > 0):

    z_t = p_{t-1} + xs_t                  TT add                (DVE)
    s_t = Sign(z_t)  in {-1,0,1}, bf16    spike                 (ScalarE ACT)
    p_t = ((z_t - [z_t>0]) + 1)*dec - 1   custom 5-stage op     (DVE)

with p_init = -1 (a single memset) and xs = x / vth prescaled on host.
The (dec - 1) shift of the state absorbs the threshold subtraction, so
each timestep costs exactly two DVE sweeps; the spike compare runs on the
otherwise-idle scalar engine.

Spikes are bit-packed on device: for each group of 8 timesteps the sign
tiles accumulate into PSUM through identity matmuls with weights
2^(k-1)*I (k = t%8) on the otherwise-idle PE; since
sum_k 2^(k-1)*(sign_k + 1) = packed_byte, the constant offset 127.5 is
added at the ACT PSUM->uint8 copy (bias).  One 128 KiB u8 DMA out per
group -> output per core [T/8, 128, 1024] u8, unpacked to bits on host.
This cuts spike-store DMA ~16x vs bf16 spikes.

Sharding: data-parallel over batch B=16 across 8 cores; the 2 local
batch items are fused along tile columns ([128, 1024] tiles; cols
[0:512] = b0, [512:1024] = b1).  vth/decay are replicated.

Measured (honest tc.For_i repeat timing, per full scan per core):
~96-106 us vs ~148 us for the previous 3-DVE-sweep kernel.
"""

import numpy as np

B, T, C, H, W = 16, 32, 64, 32, 32
VTH_M, VTH_S, DECAY_M, DECAY_S = 0.5, 0.1, 2.0, 0.1
N_CORES = 8
B_LOC = B // N_CORES          # 2 batch items per core
P = 128                       # SBUF partitions
CHW = C * H * W               # 65536 neurons
FD = CHW // P                 # 512
FDB = B_LOC * FD              # 1024 (both batch items along columns)
NG = T // 8                   # 4 bit-pack groups

_STATE: dict = {}


# --------------------------------------------------------------------------
# Custom DVE op (registered once per process)
# --------------------------------------------------------------------------

def _get_ops():
    if "ops" in _STATE:
        return _STATE["ops"]
    from concourse import dve_ops
    from concourse.dve_spec import Spec, Src0, Src1, Zero, One, lower, _has_src1
    from concourse.dve_uop import DveOpSpec

    def register(name, spec):
        for op in dve_ops.OPS:
            if op.name == name:
                return op
        row = dve_ops._CUSTOM_DVE_ROW_BASE + len(dve_ops.OPS)
        shas = {}
        for ver in ("v3", "v4"):
            s = DveOpSpec(
                name=name, opcode=row, uops=lower(spec, ver=ver),
                rd1_en=_has_src1(spec),
            )
            shas[ver] = s.sha(ver)
        op = dve_ops.DveOp(name, spec, subdim=False, uops_sha=shas)
        dve_ops.OPS.append(op)
        dve_ops._SUB_OPCODE_FOR_NAME[name] = row
        dve_ops.CUSTOM_DVE_SPECS[name] = spec
        return op

    # p' = ((z - (z>0)) + 1) * d - 1   (soft reset + leak + state shift)
    lif_pd = register(
        "LIF_PD",
        Spec(
            body=((Src0 - (Src0 > Zero)) + One) * Src1 - One,
            reference=lambda in0, in1, s0, s1, imm2: (
                ((in0.astype(np.float32) - (in0 > 0.0)) + 1.0) * in1 - 1.0
            ).astype(np.float32),
        ),
    )
    _STATE["ops"] = (lif_pd,)
    return _STATE["ops"]


# --------------------------------------------------------------------------
# Device kernel build
# --------------------------------------------------------------------------

def _emit_scan(nc, tc, pools, tiles, mybir):
    """One full T-step scan (the per-iteration body)."""
    import concourse.bass  # noqa: F401
    (lif_pd,) = _get_ops()
    f32 = mybir.dt.float32
    bf16 = mybir.dt.bfloat16
    u8 = mybir.dt.uint8
    Act = mybir.ActivationFunctionType
    xp, wp, zp, sp, up, ps = pools
    xs_d, s_d, dec, eyes, out_slot = tiles

    p = None
    psums = [None, None]
    ut = None
    for t in range(T):
        k = t % 8
        xt = xp.tile([P, FDB], f32, name=f"x{t}", tag="x")
        nc.sync.dma_start(xt, xs_d[t])
        if k == 0:
            ut = up.tile([P, FDB], u8, name=f"u{t}", tag="u")

        # DVE ops are issued in column halves: dependent ops end up two
        # issue slots apart, hiding the dependent-issue stall (~10 us/scan
        # measured vs full-width ops).  At t=0 the add is folded into the
        # host prep (xs[0] -= 1 == p_init + x), so the x tile IS z_0.
        if t == 0:
            z = xt
        else:
            z = zp.tile([P, FDB], f32, name=f"z{t}", tag="z")
            for g in range(2):
                cg = slice(g * FD, (g + 1) * FD)
                nc.vector.tensor_tensor(
                    z[:, cg], p[:, cg], xt[:, cg], mybir.AluOpType.add)

        st = sp.tile([P, FDB], bf16, name=f"s{t}", tag="s")
        nc.scalar.activation(st, z, Act.Sign)

        if t < T - 1:
            pn = wp.tile([P, FDB], f32, name=f"pn{t}", tag="p")
            for g in range(2):
                cg = slice(g * FD, (g + 1) * FD)
                nc.vector._custom_dve(
                    lif_pd, out=pn[:, cg], in0=z[:, cg], in1=dec[:, cg])
            p = pn

        for h in range(B_LOC):
            if k == 0:
                psums[h] = ps.tile([P, FD], f32, name=f"ps{t}_{h}", tag=f"ps{h}")
            nc.tensor.matmul(
                psums[h], eyes[k], st[:, h * FD:(h + 1) * FD],
                start=(k == 0), stop=(k == 7))

        if k == 7:
            for h in range(B_LOC):
                nc.scalar.activation(
                    ut[:, h * FD:(h + 1) * FD], psums[h],
                    Act.Copy, bias=127.5)
            nc.sync.dma_start(s_d[out_slot, t // 8], ut)


def _build_nc(loop_reps=None):
    """loop_reps=None: single-shot kernel.  loop_reps=R: the scan body is
    wrapped in a tc.For_i hardware loop (used only for timing)."""
    import concourse.bacc as bacc
    import concourse.mybir as mybir
    import concourse.bass as bass
    from concourse.tile import TileContext

    f32 = mybir.dt.float32
    bf16 = mybir.dt.bfloat16
    u8 = mybir.dt.uint8

    nc = bacc.Bacc(trn_type="TRN2")
    xs_d = nc.dram_tensor("xs", [T, P, FDB], f32, kind="ExternalInput")
    dec_d = nc.dram_tensor("decay2", [P, FDB], f32, kind="ExternalInput")
    eye_d = nc.dram_tensor("eye8", [8, P, P], bf16, kind="ExternalInput")
    s_d = nc.dram_tensor("spk", [1, NG, P, FDB], u8, kind="ExternalOutput")

    with TileContext(nc) as tc:
        with tc.tile_pool(name="pp", bufs=1) as pp, \
             tc.tile_pool(name="xp", bufs=6) as xp, \
             tc.tile_pool(name="wp", bufs=3) as wp, \
             tc.tile_pool(name="zp", bufs=3) as zp, \
             tc.tile_pool(name="sp", bufs=4) as sp, \
             tc.tile_pool(name="up", bufs=2) as up, \
             tc.tile_pool(name="ps", bufs=3, space=bass.MemorySpace.PSUM) as ps:

            dec = pp.tile([P, FDB], f32, name="dec", tag="dec")
            nc.sync.dma_start(dec, dec_d[:, :])
            eyes = []
            for k in range(8):
                et = pp.tile([P, P], bf16, name=f"eye{k}", tag=f"eye{k}")
                nc.sync.dma_start(et, eye_d[k])
                eyes.append(et)

            pools = (xp, wp, zp, sp, up, ps)
            tiles = (xs_d, s_d, dec, eyes, 0)
            if loop_reps is None:
                _emit_scan(nc, tc, pools, tiles, mybir)
            else:
                with tc.For_i(0, loop_reps) as _i:
                    _emit_scan(nc, tc, pools, tiles, mybir)
    nc.finalize()
    return nc


def _get_nc():
    nc = _STATE.get("nc")
    if nc is None:
        nc = _build_nc()
        _STATE["nc"] = nc
    return nc


# --------------------------------------------------------------------------
# Runner (cached jit; NEFF runs via PJRT, same path as run_bass_kernel_spmd
# under axon, but keeps the executable + device inputs reusable)
# --------------------------------------------------------------------------

def _make_runner(nc):
    import jax
    from jax.sharding import Mesh, PartitionSpec
    from jax.experimental.shard_map import shard_map
    import concourse.mybir as mybir
    from concourse import bass2jax

    bass2jax.install_neuronx_cc_hook()

    partition_name = nc.partition_id_tensor.name if nc.partition_id_tensor else None
    in_names, out_names, out_avals, zero_outs = [], [], [], []
    for alloc in nc.m.functions[0].allocations:
        if not isinstance(alloc, mybir.MemoryLocationSet):
            continue
        name = alloc.memorylocations[0].name
        if alloc.kind == "ExternalInput":
            if name != partition_name:
                in_names.append(name)
        elif alloc.kind == "ExternalOutput":
            shape = tuple(alloc.tensor_shape)
            dtype = mybir.dt.np(alloc.dtype)
            out_names.append(name)
            out_avals.append(jax.core.ShapedArray(shape, dtype))
            zero_outs.append(np.zeros(shape, dtype))
    n_params = len(in_names)
    n_outs = len(out_avals)
    all_in_names = list(in_names) + list(out_names)
    if partition_name is not None:
        all_in_names.append(partition_name)

    def _body(*args):
        operands = list(args)
        if partition_name is not None:
            operands.append(bass2jax.partition_id_tensor())
        outs = bass2jax._bass_exec_p.bind(
            *operands,
            out_avals=tuple(out_avals),
            in_names=tuple(all_in_names),
            out_names=tuple(out_names),
            lowering_input_output_aliases=(),
            sim_require_finite=True,
            sim_require_nnan=True,
            nc=nc,
        )
        return tuple(outs)

    devices = jax.devices()[:N_CORES]
    mesh = Mesh(np.asarray(devices), ("core",))
    in_specs = (PartitionSpec("core"),) * (n_params + n_outs)
    out_specs = (PartitionSpec("core"),) * n_outs
    sharded = jax.jit(
        shard_map(_body, mesh=mesh, in_specs=in_specs, out_specs=out_specs,
                  check_rep=False),
        keep_unused=True,
    )

    from jax.sharding import NamedSharding
    zero_sharding = NamedSharding(mesh, PartitionSpec("core"))
    zero_cache = []

    def run(concat_inputs_by_name):
        if not zero_cache:
            zero_cache.extend(
                jax.device_put(
                    np.zeros((N_CORES * z.shape[0], *z.shape[1:]), z.dtype),
                    zero_sharding,
                )
                for z in zero_outs
            )
        args = [concat_inputs_by_name[n] for n in in_names]
        args += zero_cache
        outs = sharded(*args)
        return outs, out_names

    run.mesh = mesh
    run.in_names = in_names
    run.out_names = out_names
    return run


# --------------------------------------------------------------------------
# Host wrapper
# --------------------------------------------------------------------------

def _prep_inputs(x, vth_raw, decay_raw):
    import ml_dtypes
    x = np.asarray(x, dtype=np.float32)
    vth_raw = np.asarray(vth_raw, dtype=np.float32)
    decay_raw = np.asarray(decay_raw, dtype=np.float32)

    vth64 = np.logaddexp(0.0, vth_raw.astype(np.float64) * VTH_S + VTH_M) + 0.01
    dec64 = 1.0 / (1.0 + np.exp(-(decay_raw.astype(np.float64) * DECAY_S + DECAY_M)))
    dec = np.clip(dec64, 0.0, 0.99).astype(np.float32)
    ivth = (1.0 / vth64).astype(np.float32)

    xs = x * ivth[None, None]                       # (B,T,C,H,W) f32
    # per-core layout [T, P, FDB]: cols [b*FD:(b+1)*FD] = batch item b
    xs_bt = xs.reshape(N_CORES, B_LOC, T, P, FD)
    xs_rs = np.ascontiguousarray(xs_bt.transpose(0, 2, 3, 1, 4))
    xs_rs = xs_rs.reshape(N_CORES, T, P, FDB)
    xs_rs[:, 0] -= 1.0        # fold p_init = -1 into t=0 (z_0 = x_0 - 1)
    dec_rs = dec.reshape(P, FD)
    dec2 = np.concatenate([dec_rs, dec_rs], axis=1)  # [P, FDB]

    eye8 = np.zeros((8, P, P), np.float32)
    for k in range(8):
        np.fill_diagonal(eye8[k], 2.0 ** (k - 1))
    eye8 = eye8.astype(ml_dtypes.bfloat16)

    return {
        "xs": xs_rs.reshape(N_CORES * T, P, FDB),
        "decay2": np.concatenate([dec2] * N_CORES, axis=0),
        "eye8": np.concatenate([eye8] * N_CORES, axis=0),
    }


def _unpack(spk_u8):
    """(N_CORES, NG, P, FDB) u8 -> (B,T,C,H,W) f32 spike tensor."""
    v = spk_u8.reshape(N_CORES, NG, P, B_LOC, FD)
    bits = np.unpackbits(v[..., None], axis=-1, bitorder="little")
    # (cores, NG, P, B_LOC, FD, 8) -> (cores, B_LOC, NG, 8, P, FD)
    bits = bits.transpose(0, 3, 1, 5, 2, 4)
    return bits.reshape(B, T, C, H, W).astype(np.float32)


def kernel(x, vth_raw, decay_raw):
    import jax
    from jax.sharding import NamedSharding, PartitionSpec

    concat = _prep_inputs(x, vth_raw, decay_raw)
    nc = _get_nc()
    run = _STATE.get("run")
    if run is None:
        run = _make_runner(nc)
        _STATE["run"] = run
    sh = NamedSharding(run.mesh, PartitionSpec("core"))
    dev_in = {n: jax.device_put(concat[n], sh) for n in run.in_names}
    outs, names = run(dev_in)
    jax.block_until_ready(outs)
    res = {n: np.asarray(o) for n, o in zip(names, outs)}
    spk = res["spk"].reshape(N_CORES, 1, NG, P, FDB)[:, 0]
    return _unpack(spk)


# --------------------------------------------------------------------------
# Honest device timing: tc.For_i hardware loop re-executes the same scan
# instructions R times (immune to dead-code elimination across repeats);
# min-wall delta between R=16 and R=1024 makes device time dominate the
# quantized transport floor of this environment.
# --------------------------------------------------------------------------

def measure_hw_ns(x, vth_raw, decay_raw, r_lo=16, r_hi=1024, n_calls=12):
    import time
    import jax
    from jax.sharding import NamedSharding, PartitionSpec

    concat = _prep_inputs(x, vth_raw, decay_raw)
    mins = {}
    for R in (r_lo, r_hi):
        nc = _build_nc(loop_reps=R)
        run = _make_runner(nc)
        sh = NamedSharding(run.mesh, PartitionSpec("core"))
        dev_in = {n: jax.device_put(concat[n], sh) for n in run.in_names}
        outs, _ = run(dev_in)
        jax.block_until_ready(outs)
        ts = []
        for _ in range(n_calls):
            t0 = time.perf_counter()
            outs, _ = run(dev_in)
            jax.block_until_ready(outs)
            ts.append(time.perf_counter() - t0)
        mins[R] = min(ts)
        print(f"  R={R}: min={min(ts)*1e3:.2f} ms  "
              f"all={[f'{t*1e3:.1f}' for t in sorted(ts)]}")
    ns = (mins[r_hi] - mins[r_lo]) / (r_hi - r_lo) * 1e9
    return ns, mins
